# revision 22
# baseline (speedup 1.0000x reference)
"""Trainium2 Bass kernel for causal multi-head attention (b=2, n=2048, d=1024, h=16).

Sharding: 8 cores; core c handles batch (c // 4) and the 4 heads
[4*(c%4), 4*(c%4)+4).  Each core computes its heads' attention plus its
partial output projection y_part = O_heads @ Wo[:, cols].T ; the host sums
the per-batch partials and adds bo (with the V-bias contribution folded in
host-side: softmax rows sum to 1, so bv contributes exactly bv @ Wo.T).

All matmul operands are bf16 (fp32 PSUM accumulation); inputs are converted
host-side, halving the startup DMA vs f32r and enabling fast weight load.

Structure: the scalar-engine exp is the pacer of the attention phase, so the
projection matmuls for block b+1 and the output projection of block q-1 are
woven INTO block q's attention stream as PE filler units — the PE works
through fillers while ACT chews exp tiles.  Startup is DMA-chased: the
merged wqkv weight and x chunks stream in k-chunk order and the first
projection matmuls wait on per-chunk DMA semaphores (no junk warm-up).

Per-core pipeline:
  xT (d-major, bf16) -> QTz, KT [hd, n] and V [n, hd] projections (bf16)
  ST tile [k,q] = KT-chunk.T x QTz_h       (K=128: both heads' KT rows with
                                            the other head's QT rows zeroed;
                                            1/8 scale folded into QT)
  PT = exp(ST) in bf16 (no max subtraction; scores are O(10), fp32-exp safe)
  causal masking: the diagonal 128-col triangle of each diagonal chunk is
  multiplied with a single [128,128] 0/1 bf16 mask block
  OT_aug [65, q] += V_aug-chunk.T x PT     (V_aug = [V | ones]; row 64 = l)
  normalize per head: l copied off PSUM (DVE), reciprocal (DVE), broadcast
  across 64 partitions on the otherwise-idle GpSimd engine
  (partition_broadcast), one fused PSUM-read mul into the OTn2 pair tile
  (heads 2p, 2p+1 stacked -> out-proj K=128 with no zero rows)
  y[tok, :] += OTn2-pair-chunk.T x Wo_pair ; y is bf16 (host sums partials)
  last q-block: pair-0's output projection runs during pair-1's attention
  (separate y1 partial tensor) so the tail only waits on the final pair.
"""

import numpy as np

import concourse.bass as bass
import concourse.mybir as mybir
import concourse.tile as tile
from concourse import bacc
from concourse.bass_utils import run_bass_kernel_spmd

D = 1024          # d_model
N = 2048          # sequence length
B = 2             # batch
H_TOT = 16        # total heads
HD = 64           # head dim
HPC = 4           # heads per core
NCORES = 8
SCALE = HD ** -0.5

F32 = mybir.dt.float32
BF16 = mybir.dt.bfloat16

QTILE = 512       # q-tile width (free dim of score matmuls)
KCH = 128         # k-chunk (partition dim of score tiles)
NQT = N // QTILE  # 4
DCH = D // 128    # 8 d_model chunks
VROW = HD + 1     # V columns per head incl. ones column
WQ0, WK0, WV0 = 0, HPC * HD, 2 * HPC * HD   # col offsets in merged wqkv


def build_kernel():
    nc = bacc.Bacc("TRN2", target_bir_lowering=False, debug=False,
                   num_devices=NCORES)

    xT = nc.dram_tensor("xT", [D, N], BF16, kind="ExternalInput").ap()
    wqkv = nc.dram_tensor("wqkvT", [D, 3 * HPC * HD], BF16,
                          kind="ExternalInput").ap()
    wo = nc.dram_tensor("woT", [HPC * HD, D], BF16, kind="ExternalInput").ap()
    bqz = nc.dram_tensor("bqz", [128, 2], F32, kind="ExternalInput").ap()
    bkd = nc.dram_tensor("bk", [HPC * HD], F32, kind="ExternalInput").ap()
    maskd = nc.dram_tensor("mask", [128, 128], BF16, kind="ExternalInput").ap()
    y = nc.dram_tensor("y", [N, D], BF16, kind="ExternalOutput").ap()
    # pair-1 partial of the last q-block (host adds it)
    y1 = nc.dram_tensor("y1", [QTILE, D], BF16, kind="ExternalOutput").ap()

    Exp = mybir.ActivationFunctionType.Exp
    Identity = mybir.ActivationFunctionType.Identity

    with tile.TileContext(nc) as tc:
        from collections import deque
        from contextlib import ExitStack
        with ExitStack() as ctx:
            singles = ctx.enter_context(tc.tile_pool(name="singles", bufs=1))
            pt_pool = ctx.enter_context(tc.tile_pool(name="pt", bufs=6))
            r_pool = ctx.enter_context(tc.tile_pool(name="rp", bufs=2))
            yout = ctx.enter_context(tc.tile_pool(name="yout", bufs=3))
            ps_mm = ctx.enter_context(
                tc.tile_pool(name="psmm", bufs=2, space="PSUM"))
            ps_st = ctx.enter_context(
                tc.tile_pool(name="psst", bufs=2, space="PSUM"))
            ps_ot = ctx.enter_context(
                tc.tile_pool(name="psot", bufs=2, space="PSUM"))

            # --- resident tiles -------------------------------------------
            bqz_sb = singles.tile([128, 2], F32)
            bk_sb = singles.tile([128, 2], F32)
            mask_sb = singles.tile([128, 128], BF16)
            wqkv_sb = singles.tile([128, DCH, 3 * HPC * HD], BF16)
            xk = [[singles.tile([128, QTILE], BF16, name=f"xk{k}_{b}")
                   for b in range(NQT)] for k in range(DCH)]
            wo2 = [singles.tile([128, D], BF16, name=f"wo{p}")
                   for p in range(2)]
            QT2 = [[singles.tile([128, QTILE], BF16, name=f"qt{m}_{i}")
                    for i in range(NQT)] for m in range(2)]
            KT_sb = [singles.tile([128, 2, QTILE], BF16, name=f"kt{i}")
                     for i in range(NQT)]
            V_sb = [singles.tile([128, 4, HPC, VROW], BF16, name=f"v{i}")
                    for i in range(NQT)]
            # OTn2[p]: heads 2p (rows 0..63) and 2p+1 (rows 64..127) stacked
            OTn2 = [singles.tile([128, N], BF16, name=f"otn{p}")
                    for p in range(2)]

            # --- DMA stream (order = priority; compute chases it) ---------
            for k in range(DCH):
                r = slice(k * 128, (k + 1) * 128)
                nc.sync.dma_start(wqkv_sb[:, k, :], wqkv[r, :])
                nc.sync.dma_start(xk[k][0][:], xT[r, 0:QTILE])
                if k == 1:
                    nc.sync.dma_start(bqz_sb[:], bqz)
                    nc.sync.dma_start(bk_sb[:],
                                      bkd.rearrange("(o p) -> p o", p=128))
            nc.sync.dma_start(mask_sb[:], maskd)
            for k in range(DCH):
                nc.sync.dma_start(xk[k][1][:],
                                  xT[k * 128:(k + 1) * 128, QTILE:2 * QTILE])
            for p in range(2):
                nc.sync.dma_start(wo2[p][:], wo[p * 128:(p + 1) * 128, :])
            for b in range(2, NQT):
                for k in range(DCH):
                    nc.sync.dma_start(
                        xk[k][b][:],
                        xT[k * 128:(k + 1) * 128,
                           b * QTILE:(b + 1) * QTILE])

            # --- PE filler units ------------------------------------------
            fillers = deque()

            def pump(n=1):
                for _ in range(n):
                    if not fillers:
                        return
                    fillers.popleft()()

            def q_unit(blk, m):
                ps = ps_mm.tile([128, 512], F32, tag="mm", name="psq")
                for k in range(DCH):
                    nc.tensor.matmul(
                        ps[:],
                        lhsT=wqkv_sb[:, k, WQ0 + m * 128:WQ0 + (m + 1) * 128],
                        rhs=xk[k][blk][:],
                        start=(k == 0), stop=(k == DCH - 1))
                nc.vector.tensor_scalar(
                    QT2[m][blk][:], ps[:], SCALE, bqz_sb[:, m:m + 1],
                    mybir.AluOpType.mult, mybir.AluOpType.add)

            def k_unit(blk, m):
                ps = ps_mm.tile([128, 512], F32, tag="mm", name="psk")
                for k in range(DCH):
                    nc.tensor.matmul(
                        ps[:],
                        lhsT=wqkv_sb[:, k, WK0 + m * 128:WK0 + (m + 1) * 128],
                        rhs=xk[k][blk][:],
                        start=(k == 0), stop=(k == DCH - 1))
                nc.vector.tensor_scalar_add(
                    KT_sb[blk][:, m, :], ps[:], bk_sb[:, m:m + 1])

            def v_unit(blk, tt):
                if tt == 0:
                    nc.vector.memset(V_sb[blk][:, :, :, HD], 1.0)
                ps = ps_mm.tile([128, 512], F32, tag="mm", name="psv")
                for k in range(DCH):
                    nc.tensor.matmul(
                        ps[:, :HPC * HD],
                        lhsT=xk[k][blk][:, tt * 128:(tt + 1) * 128],
                        rhs=wqkv_sb[:, k, WV0:WV0 + HPC * HD],
                        start=(k == 0), stop=(k == DCH - 1))
                nc.scalar.activation(
                    V_sb[blk][:, tt, :, :HD], ps[:, :HPC * HD], Identity,
                    bias=0.0, scale=1.0)

            def outproj_unit(qi, tt, pairs=(0, 1), ydst=None, yrow0=0):
                t0 = qi * QTILE + tt * 128
                yt = yout.tile([128, 2, 512], BF16, tag="y", name="yt")
                for half in range(2):
                    ps = ps_mm.tile([128, 512], F32, tag="mm", name="psy")
                    for i, p in enumerate(pairs):
                        nc.tensor.matmul(
                            ps[:], lhsT=OTn2[p][:, t0:t0 + 128],
                            rhs=wo2[p][:, half * 512:half * 512 + 512],
                            start=(i == 0), stop=(i == len(pairs) - 1))
                    nc.vector.tensor_copy(yt[:, half, :], ps[:])
                dst = y if ydst is None else ydst
                nc.sync.dma_start(dst[t0 - yrow0:t0 - yrow0 + 128, :], yt[:])

            # --- attention: one head PAIR at a time; score matmuls for
            # the two heads run CONCURRENTLY as 64-row tiles T0/T8 (the
            # q/k projections put head 2m on partitions 0-63 and head
            # 2m+1 on 64-127, so no zero-padding is needed).  PV matmuls
            # are full 128-mode; chunks alternate modes per 2-chunk group
            # (~105ns/switch, far less than the 2x score-matmul saving).
            def attention_pair(qi, hp):
                q0 = qi * QTILE
                mi = hp
                nch = 4 * (qi + 1)
                pso_a = ps_ot.tile([VROW, 512], F32, tag="ot", name="psoa")
                pso_b = ps_ot.tile([VROW, 512], F32, tag="ot", name="psob")
                psos = (pso_a, pso_b)
                chunk_pt = [None] * nch

                def mask_pv(c, hh):
                    pt = chunk_pt[c]
                    r = c * KCH - q0
                    if r >= 0:
                        nc.vector.tensor_mul(
                            pt[:, hh, r:r + KCH], pt[:, hh, r:r + KCH],
                            mask_sb[:])
                    s = max(r, 0)
                    nc.tensor.matmul(
                        psos[hh][:, s:],
                        lhsT=V_sb[c // 4][:, c % 4, 2 * hp + hh, :],
                        rhs=pt[:, hh, s:],
                        start=(c == 0), stop=(c == nch - 1))

                def consume(c):
                    for hh in range(2):
                        mask_pv(c, hh)

                for pi in range(nch // 2):
                    for j in range(2):
                        c = 2 * pi + j
                        s = (c * KCH - q0) if c >= nch - 2 else 0
                        pss = ps_st.tile([128, 2, 512], F32, tag="st",
                                         name="pss")
                        pt = pt_pool.tile([128, 2, 512], BF16, tag="pt",
                                          name="pt")
                        chunk_pt[c] = pt
                        kslc = KT_sb[c // 4][:, mi,
                                             (c % 4) * 128:(c % 4) * 128 + 128]
                        qt = QT2[mi][qi]
                        nc.tensor.matmul(
                            pss[:, 0, s:], lhsT=kslc[0:64, :],
                            rhs=qt[0:64, s:], start=True, stop=True)
                        nc.tensor.matmul(
                            pss[:, 1, s:], lhsT=kslc[64:128, :],
                            rhs=qt[64:128, s:], start=True, stop=True)
                        for hh in range(2):
                            nc.scalar.activation(
                                pt[:, hh, s:], pss[:, hh, s:], Exp)
                    if pi > 1:
                        consume(2 * pi - 4)
                        consume(2 * pi - 3)
                    pump(1)
                consume(nch - 4)
                consume(nch - 3)
                # tail: finish head A entirely, normalize it while head
                # B's last PV matmuls and normalize run
                mask_pv(nch - 2, 0)
                mask_pv(nch - 1, 0)
                normalize_head(qi, 2 * hp, pso_a)
                mask_pv(nch - 2, 1)
                mask_pv(nch - 1, 1)
                normalize_head(qi, 2 * hp + 1, pso_b)

            def normalize_head(qi, h, pso):
                q0 = qi * QTILE
                p, po = h // 2, (h % 2) * HD
                lw = r_pool.tile([1, 512], F32, tag="lw", name="lw1")
                nc.vector.tensor_copy(lw[:], pso[HD:HD + 1, :])
                rl = r_pool.tile([1, 512], F32, tag="rl", name="rl1")
                nc.vector.reciprocal_approx_fast(out=rl[:], in_=lw[:])
                rb = r_pool.tile([HD, 512], F32, tag="rb", name="rb1")
                nc.gpsimd.partition_broadcast(rb[:], rl[:])
                nc.vector.tensor_mul(
                    OTn2[p][po:po + HD, q0:q0 + QTILE], pso[:HD, :], rb[:])

            # --- schedule -------------------------------------------------
            # block-0 Q/K projections, emitted k-major across four open
            # PSUM groups so the matmuls chase the per-chunk DMA arrivals
            # at full duty (ps_st/ps_ot are idle this early — borrow them)
            ps_q0 = ps_mm.tile([128, 512], F32, tag="mm", name="psq0")
            ps_q1 = ps_mm.tile([128, 512], F32, tag="mm", name="psq1")
            ps_k0 = ps_st.tile([128, 512], F32, tag="st", name="psk0")
            ps_k1 = ps_ot.tile([128, 512], F32, tag="ot", name="psk1")
            for k in range(DCH):
                st, sp = (k == 0), (k == DCH - 1)
                for m, ps in ((0, ps_q0), (1, ps_q1)):
                    nc.tensor.matmul(
                        ps[:],
                        lhsT=wqkv_sb[:, k, WQ0 + m * 128:WQ0 + (m + 1) * 128],
                        rhs=xk[k][0][:], start=st, stop=sp)
                for m, ps in ((0, ps_k0), (1, ps_k1)):
                    nc.tensor.matmul(
                        ps[:],
                        lhsT=wqkv_sb[:, k, WK0 + m * 128:WK0 + (m + 1) * 128],
                        rhs=xk[k][0][:], start=st, stop=sp)
            # post-process on ACT (idle this early), first-needed first:
            # head 0's attention needs QTz[0] and KT m0 before anything else
            nc.scalar.activation(QT2[0][0][:], ps_q0[:], Identity,
                                 bias=bqz_sb[:, 0:1], scale=SCALE)
            nc.scalar.activation(KT_sb[0][:, 0, :], ps_k0[:], Identity,
                                 bias=bk_sb[:, 0:1], scale=1.0)
            nc.scalar.activation(QT2[1][0][:], ps_q1[:], Identity,
                                 bias=bqz_sb[:, 1:2], scale=SCALE)
            nc.scalar.activation(KT_sb[0][:, 1, :], ps_k1[:], Identity,
                                 bias=bk_sb[:, 1:2], scale=1.0)
            for tt in range(4):
                v_unit(0, tt)

            for qi in range(NQT):
                # queue fillers: next block's projections; output
                # projections run two blocks later, where the scalar
                # engine (exp) is the pacer and the PE has slack
                if qi + 1 < NQT:
                    for m in range(2):
                        fillers.append(
                            lambda b=qi + 1, m=m: q_unit(b, m))
                        fillers.append(
                            lambda b=qi + 1, m=m: k_unit(b, m))
                if qi >= 2:
                    for tt in range(4):
                        fillers.append(
                            lambda q=qi - 2, t=tt: outproj_unit(q, t))
                if qi == NQT - 1:
                    for tt in range(4):
                        fillers.append(
                            lambda q=qi - 1, t=tt: outproj_unit(q, t))
                if qi + 1 < NQT:
                    for tt in range(4):
                        fillers.append(
                            lambda b=qi + 1, t=tt: v_unit(b, t))

                last = (qi == NQT - 1)
                for hp in range(2):
                    attention_pair(qi, hp)
                    if last and hp == 0:
                        # pair-0 output projection of the last block runs
                        # during pair-1's attention, into its own partial
                        for tt in range(4):
                            fillers.append(
                                lambda t=tt:
                                outproj_unit(NQT - 1, t, pairs=(0,)))
                # all remaining fillers must land before the next block's
                # attention (it needs the projections)
                pump(len(fillers))

            # tail: pair-1 output projection of the last block
            for tt in range(4):
                outproj_unit(NQT - 1, tt, pairs=(1,), ydst=y1,
                             yrow0=(NQT - 1) * QTILE)

    nc.compile()
    return nc


def make_in_maps(x, Wq, bq, Wkv, bkv, Wo, bo):
    import ml_dtypes

    bf = ml_dtypes.bfloat16
    x = np.asarray(x, np.float32)
    Wq = np.asarray(Wq, np.float32)
    bq = np.asarray(bq, np.float32)
    Wkv = np.asarray(Wkv, np.float32)
    bkv = np.asarray(bkv, np.float32)
    Wo = np.asarray(Wo, np.float32)

    Wk, Wv = Wkv[:D], Wkv[D:]
    bk, bv = bkv[:D], bkv[D:]

    # causal triangle keep-mask: mask[p, u] = 1 iff u >= p
    u = np.arange(128)[None, :]
    kk = np.arange(128)[:, None]
    mask = (u >= kk).astype(bf)

    in_maps = []
    for c in range(NCORES):
        b = c // (NCORES // B)
        hs = HPC * (c % (NCORES // B))
        rows = slice(hs * HD, hs * HD + HPC * HD)
        bq_c = bq[rows] * SCALE
        # bqz column m = the scaled q-bias for heads (2m, 2m+1) stacked
        bqzv = np.zeros((128, 2), np.float32)
        for m in range(2):
            bqzv[:, m] = bq_c[m * 128:(m + 1) * 128]
        wqkv_c = np.concatenate(
            [Wq[rows].T, Wk[rows].T, Wv[rows].T], axis=1)
        in_maps.append({
            "xT": np.ascontiguousarray(x[b].T).astype(bf),
            "wqkvT": np.ascontiguousarray(wqkv_c).astype(bf),
            "woT": np.ascontiguousarray(Wo[:, rows].T).astype(bf),
            "bqz": bqzv,
            "bk": np.ascontiguousarray(bk[rows]),
            "mask": mask,
        })
    return in_maps


_NC_CACHE = None


def _get_nc():
    global _NC_CACHE
    if _NC_CACHE is None:
        _NC_CACHE = build_kernel()
    return _NC_CACHE


def kernel(x, Wq, bq, Wkv, bkv, Wo, bo, _trace=False, _trace_kwargs=None):
    nc = _get_nc()
    in_maps = make_in_maps(x, Wq, bq, Wkv, bkv, Wo, bo)
    kwargs = {}
    if _trace:
        kwargs = dict(trace=True, trace_cores=list(range(NCORES)),
                      **(_trace_kwargs or {}))
    res = run_bass_kernel_spmd(nc, in_maps, core_ids=list(range(NCORES)),
                               **kwargs)
    out = np.zeros((B, N, D), np.float32)
    for c, r in enumerate(res.results):
        b = c // (NCORES // B)
        out[b] += np.asarray(r["y"], np.float32)
        out[b, (NQT - 1) * QTILE:] += np.asarray(r["y1"], np.float32)
    # bo plus the folded V-bias contribution (softmax rows sum to 1, so the
    # v-bias adds exactly bv @ Wo.T to every token)
    bv = np.asarray(bkv, np.float32)[D:]
    bo2 = np.asarray(bo, np.float32) + np.asarray(Wo, np.float32) @ bv
    out += bo2[None, None, :]
    if _trace:
        kernel.last_results = res
    return out


# revision 23
# speedup vs baseline: 1.0142x; 1.0142x over previous
"""Trainium2 Bass kernel for causal multi-head attention (b=2, n=2048, d=1024, h=16).

Sharding: 8 cores; core c handles batch (c // 4) and the 4 heads
[4*(c%4), 4*(c%4)+4).  Each core computes its heads' attention plus its
partial output projection y_part = O_heads @ Wo[:, cols].T ; the host sums
the per-batch partials and adds bo (with the V-bias contribution folded in
host-side: softmax rows sum to 1, so bv contributes exactly bv @ Wo.T).

All matmul operands are bf16 (fp32 PSUM accumulation); inputs are converted
host-side, halving the startup DMA vs f32r and enabling fast weight load.

Structure: the scalar-engine exp is the pacer of the attention phase, so the
projection matmuls for block b+1 and the output projection of block q-1 are
woven INTO block q's attention stream as PE filler units — the PE works
through fillers while ACT chews exp tiles.  Startup is DMA-chased: the
merged wqkv weight and x chunks stream in k-chunk order and the first
projection matmuls wait on per-chunk DMA semaphores (no junk warm-up).

Per-core pipeline:
  xT (d-major, bf16) -> QTz, KT [hd, n] and V [n, hd] projections (bf16)
  ST tile [k,q] = KT-chunk.T x QTz_h       (K=128: both heads' KT rows with
                                            the other head's QT rows zeroed;
                                            1/8 scale folded into QT)
  PT = exp(ST) in bf16 (no max subtraction; scores are O(10), fp32-exp safe)
  causal masking: the diagonal 128-col triangle of each diagonal chunk is
  multiplied with a single [128,128] 0/1 bf16 mask block
  OT_aug [65, q] += V_aug-chunk.T x PT     (V_aug = [V | ones]; row 64 = l)
  normalize per head: l copied off PSUM (DVE), reciprocal (DVE), broadcast
  across 64 partitions on the otherwise-idle GpSimd engine
  (partition_broadcast), one fused PSUM-read mul into the OTn2 pair tile
  (heads 2p, 2p+1 stacked -> out-proj K=128 with no zero rows)
  y[tok, :] += OTn2-pair-chunk.T x Wo_pair ; y is bf16 (host sums partials)
  last q-block: pair-0's output projection runs during pair-1's attention
  (separate y1 partial tensor) so the tail only waits on the final pair.
"""

import numpy as np

import concourse.bass as bass
import concourse.mybir as mybir
import concourse.tile as tile
from concourse import bacc
from concourse.bass_utils import run_bass_kernel_spmd

D = 1024          # d_model
N = 2048          # sequence length
B = 2             # batch
H_TOT = 16        # total heads
HD = 64           # head dim
HPC = 4           # heads per core
NCORES = 8
SCALE = HD ** -0.5

F32 = mybir.dt.float32
BF16 = mybir.dt.bfloat16

QTILE = 512       # q-tile width (free dim of score matmuls)
KCH = 128         # k-chunk (partition dim of score tiles)
NQT = N // QTILE  # 4
DCH = D // 128    # 8 d_model chunks
VROW = HD + 1     # V columns per head incl. ones column
WQ0, WK0, WV0 = 0, HPC * HD, 2 * HPC * HD   # col offsets in merged wqkv


def build_kernel():
    nc = bacc.Bacc("TRN2", target_bir_lowering=False, debug=False,
                   num_devices=NCORES)

    xT = nc.dram_tensor("xT", [D, N], BF16, kind="ExternalInput").ap()
    wqkv = nc.dram_tensor("wqkvT", [D, 3 * HPC * HD], BF16,
                          kind="ExternalInput").ap()
    wo = nc.dram_tensor("woT", [HPC * HD, D], BF16, kind="ExternalInput").ap()
    bqz = nc.dram_tensor("bqz", [128, 2], F32, kind="ExternalInput").ap()
    bkd = nc.dram_tensor("bk", [HPC * HD], F32, kind="ExternalInput").ap()
    maskd = nc.dram_tensor("mask", [128, 128], BF16, kind="ExternalInput").ap()
    y = nc.dram_tensor("y", [N, D], BF16, kind="ExternalOutput").ap()
    # pair-1 partial of the last q-block (host adds it)
    y1 = nc.dram_tensor("y1", [QTILE, D], BF16, kind="ExternalOutput").ap()

    Exp = mybir.ActivationFunctionType.Exp
    Identity = mybir.ActivationFunctionType.Identity

    with tile.TileContext(nc) as tc:
        from collections import deque
        from contextlib import ExitStack
        with ExitStack() as ctx:
            singles = ctx.enter_context(tc.tile_pool(name="singles", bufs=1))
            pt_pool = ctx.enter_context(tc.tile_pool(name="pt", bufs=6))
            r_pool = ctx.enter_context(tc.tile_pool(name="rp", bufs=2))
            yout = ctx.enter_context(tc.tile_pool(name="yout", bufs=3))
            ps_mm = ctx.enter_context(
                tc.tile_pool(name="psmm", bufs=2, space="PSUM"))
            ps_st = ctx.enter_context(
                tc.tile_pool(name="psst", bufs=2, space="PSUM"))
            ps_ot = ctx.enter_context(
                tc.tile_pool(name="psot", bufs=2, space="PSUM"))

            # --- resident tiles -------------------------------------------
            bqz_sb = singles.tile([128, 2], F32)
            bk_sb = singles.tile([128, 2], F32)
            mask_sb = singles.tile([128, 128], BF16)
            wqkv_sb = singles.tile([128, DCH, 3 * HPC * HD], BF16)
            xk = [[singles.tile([128, QTILE], BF16, name=f"xk{k}_{b}")
                   for b in range(NQT)] for k in range(DCH)]
            wo2 = [singles.tile([128, D], BF16, name=f"wo{p}")
                   for p in range(2)]
            QT2 = [[singles.tile([128, QTILE], BF16, name=f"qt{m}_{i}")
                    for i in range(NQT)] for m in range(2)]
            KT_sb = [singles.tile([128, 2, QTILE], BF16, name=f"kt{i}")
                     for i in range(NQT)]
            # each head's V region padded to 128 cols so the PV matmul
            # is full 128x128 mode (M=128; rows 65-127 accumulate unread
            # garbage) — keeps the HAM activity monitor fed
            V_sb = [singles.tile([128, 4, HPC, 128], BF16, name=f"v{i}")
                    for i in range(NQT)]
            # OTn2[p]: heads 2p (rows 0..63) and 2p+1 (rows 64..127) stacked
            OTn2 = [singles.tile([128, N], BF16, name=f"otn{p}")
                    for p in range(2)]

            # --- DMA stream (order = priority; compute chases it) ---------
            for k in range(DCH):
                r = slice(k * 128, (k + 1) * 128)
                nc.sync.dma_start(wqkv_sb[:, k, :], wqkv[r, :])
                nc.sync.dma_start(xk[k][0][:], xT[r, 0:QTILE])
                if k == 1:
                    nc.sync.dma_start(bqz_sb[:], bqz)
                    nc.sync.dma_start(bk_sb[:],
                                      bkd.rearrange("(o p) -> p o", p=128))
            nc.sync.dma_start(mask_sb[:], maskd)
            for k in range(DCH):
                nc.sync.dma_start(xk[k][1][:],
                                  xT[k * 128:(k + 1) * 128, QTILE:2 * QTILE])
            for p in range(2):
                nc.sync.dma_start(wo2[p][:], wo[p * 128:(p + 1) * 128, :])
            for b in range(2, NQT):
                for k in range(DCH):
                    nc.sync.dma_start(
                        xk[k][b][:],
                        xT[k * 128:(k + 1) * 128,
                           b * QTILE:(b + 1) * QTILE])

            # --- PE filler units ------------------------------------------
            fillers = deque()

            def pump(n=1):
                for _ in range(n):
                    if not fillers:
                        return
                    fillers.popleft()()

            def q_unit(blk, m):
                ps = ps_mm.tile([128, 512], F32, tag="mm", name="psq")
                for k in range(DCH):
                    nc.tensor.matmul(
                        ps[:],
                        lhsT=wqkv_sb[:, k, WQ0 + m * 128:WQ0 + (m + 1) * 128],
                        rhs=xk[k][blk][:],
                        start=(k == 0), stop=(k == DCH - 1))
                nc.vector.tensor_scalar(
                    QT2[m][blk][:], ps[:], SCALE, bqz_sb[:, m:m + 1],
                    mybir.AluOpType.mult, mybir.AluOpType.add)

            def k_unit(blk, m):
                ps = ps_mm.tile([128, 512], F32, tag="mm", name="psk")
                for k in range(DCH):
                    nc.tensor.matmul(
                        ps[:],
                        lhsT=wqkv_sb[:, k, WK0 + m * 128:WK0 + (m + 1) * 128],
                        rhs=xk[k][blk][:],
                        start=(k == 0), stop=(k == DCH - 1))
                nc.vector.tensor_scalar_add(
                    KT_sb[blk][:, m, :], ps[:], bk_sb[:, m:m + 1])

            def v_unit(blk, tt):
                if tt == 0:
                    nc.vector.memset(V_sb[blk][:, :, :, HD], 1.0)
                ps = ps_mm.tile([128, 512], F32, tag="mm", name="psv")
                for k in range(DCH):
                    nc.tensor.matmul(
                        ps[:, :HPC * HD],
                        lhsT=xk[k][blk][:, tt * 128:(tt + 1) * 128],
                        rhs=wqkv_sb[:, k, WV0:WV0 + HPC * HD],
                        start=(k == 0), stop=(k == DCH - 1))
                nc.scalar.activation(
                    V_sb[blk][:, tt, :, :HD], ps[:, :HPC * HD], Identity,
                    bias=0.0, scale=1.0)

            def outproj_unit(qi, tt, pairs=(0, 1), ydst=None, yrow0=0):
                t0 = qi * QTILE + tt * 128
                yt = yout.tile([128, 2, 512], BF16, tag="y", name="yt")
                for half in range(2):
                    ps = ps_mm.tile([128, 512], F32, tag="mm", name="psy")
                    for i, p in enumerate(pairs):
                        nc.tensor.matmul(
                            ps[:], lhsT=OTn2[p][:, t0:t0 + 128],
                            rhs=wo2[p][:, half * 512:half * 512 + 512],
                            start=(i == 0), stop=(i == len(pairs) - 1))
                    nc.vector.tensor_copy(yt[:, half, :], ps[:])
                dst = y if ydst is None else ydst
                nc.sync.dma_start(dst[t0 - yrow0:t0 - yrow0 + 128, :], yt[:])

            # --- attention: one head PAIR at a time; score matmuls for
            # the two heads run CONCURRENTLY as 64-row tiles T0/T8 (the
            # q/k projections put head 2m on partitions 0-63 and head
            # 2m+1 on 64-127, so no zero-padding is needed).  PV matmuls
            # are full 128-mode; chunks alternate modes per 2-chunk group
            # (~105ns/switch, far less than the 2x score-matmul saving).
            def attention_pair(qi, hp):
                q0 = qi * QTILE
                mi = hp
                nch = 4 * (qi + 1)
                pso_a = ps_ot.tile([128, 512], F32, tag="ot", name="psoa")
                pso_b = ps_ot.tile([128, 512], F32, tag="ot", name="psob")
                psos = (pso_a, pso_b)
                chunk_pt = [None] * nch

                def mask_pv(c, hh):
                    pt = chunk_pt[c]
                    r = c * KCH - q0
                    if r >= 0:
                        nc.vector.tensor_mul(
                            pt[:, hh, r:r + KCH], pt[:, hh, r:r + KCH],
                            mask_sb[:])
                    s = max(r, 0)
                    nc.tensor.matmul(
                        psos[hh][:, s:],
                        lhsT=V_sb[c // 4][:, c % 4, 2 * hp + hh, :],
                        rhs=pt[:, hh, s:],
                        start=(c == 0), stop=(c == nch - 1))

                def consume(c):
                    for hh in range(2):
                        mask_pv(c, hh)

                for pi in range(nch // 2):
                    for j in range(2):
                        c = 2 * pi + j
                        s = (c * KCH - q0) if c >= nch - 2 else 0
                        pss = ps_st.tile([128, 2, 512], F32, tag="st",
                                         name="pss")
                        pt = pt_pool.tile([128, 2, 512], BF16, tag="pt",
                                          name="pt")
                        chunk_pt[c] = pt
                        kslc = KT_sb[c // 4][:, mi,
                                             (c % 4) * 128:(c % 4) * 128 + 128]
                        qt = QT2[mi][qi]
                        nc.tensor.matmul(
                            pss[:, 0, s:], lhsT=kslc[0:64, :],
                            rhs=qt[0:64, s:], start=True, stop=True)
                        nc.tensor.matmul(
                            pss[:, 1, s:], lhsT=kslc[64:128, :],
                            rhs=qt[64:128, s:], start=True, stop=True)
                        for hh in range(2):
                            nc.scalar.activation(
                                pt[:, hh, s:], pss[:, hh, s:], Exp)
                    if pi > 1:
                        consume(2 * pi - 4)
                        consume(2 * pi - 3)
                    pump(1)
                consume(nch - 4)
                consume(nch - 3)
                # tail: finish head A entirely, normalize it while head
                # B's last PV matmuls and normalize run
                mask_pv(nch - 2, 0)
                mask_pv(nch - 1, 0)
                normalize_head(qi, 2 * hp, pso_a)
                mask_pv(nch - 2, 1)
                mask_pv(nch - 1, 1)
                normalize_head(qi, 2 * hp + 1, pso_b)

            def normalize_head(qi, h, pso):
                q0 = qi * QTILE
                p, po = h // 2, (h % 2) * HD
                lw = r_pool.tile([1, 512], F32, tag="lw", name="lw1")
                nc.vector.tensor_copy(lw[:], pso[HD:HD + 1, :])
                rl = r_pool.tile([1, 512], F32, tag="rl", name="rl1")
                nc.vector.reciprocal_approx_fast(out=rl[:], in_=lw[:])
                rb = r_pool.tile([HD, 512], F32, tag="rb", name="rb1")
                nc.gpsimd.partition_broadcast(rb[:], rl[:])
                nc.vector.tensor_mul(
                    OTn2[p][po:po + HD, q0:q0 + QTILE], pso[:HD, :], rb[:])

            # --- schedule -------------------------------------------------
            # block-0 Q/K projections, emitted k-major across four open
            # PSUM groups so the matmuls chase the per-chunk DMA arrivals
            # at full duty (ps_st/ps_ot are idle this early — borrow them)
            ps_q0 = ps_mm.tile([128, 512], F32, tag="mm", name="psq0")
            ps_q1 = ps_mm.tile([128, 512], F32, tag="mm", name="psq1")
            ps_k0 = ps_st.tile([128, 512], F32, tag="st", name="psk0")
            ps_k1 = ps_ot.tile([128, 512], F32, tag="ot", name="psk1")
            for k in range(DCH):
                st, sp = (k == 0), (k == DCH - 1)
                for m, ps in ((0, ps_q0), (1, ps_q1)):
                    nc.tensor.matmul(
                        ps[:],
                        lhsT=wqkv_sb[:, k, WQ0 + m * 128:WQ0 + (m + 1) * 128],
                        rhs=xk[k][0][:], start=st, stop=sp)
                for m, ps in ((0, ps_k0), (1, ps_k1)):
                    nc.tensor.matmul(
                        ps[:],
                        lhsT=wqkv_sb[:, k, WK0 + m * 128:WK0 + (m + 1) * 128],
                        rhs=xk[k][0][:], start=st, stop=sp)
            # post-process on ACT (idle this early), first-needed first:
            # head 0's attention needs QTz[0] and KT m0 before anything else
            nc.scalar.activation(QT2[0][0][:], ps_q0[:], Identity,
                                 bias=bqz_sb[:, 0:1], scale=SCALE)
            nc.scalar.activation(KT_sb[0][:, 0, :], ps_k0[:], Identity,
                                 bias=bk_sb[:, 0:1], scale=1.0)
            nc.scalar.activation(QT2[1][0][:], ps_q1[:], Identity,
                                 bias=bqz_sb[:, 1:2], scale=SCALE)
            nc.scalar.activation(KT_sb[0][:, 1, :], ps_k1[:], Identity,
                                 bias=bk_sb[:, 1:2], scale=1.0)
            for tt in range(4):
                v_unit(0, tt)

            for qi in range(NQT):
                # queue fillers: next block's projections; output
                # projections run two blocks later, where the scalar
                # engine (exp) is the pacer and the PE has slack
                if qi + 1 < NQT:
                    for m in range(2):
                        fillers.append(
                            lambda b=qi + 1, m=m: q_unit(b, m))
                        fillers.append(
                            lambda b=qi + 1, m=m: k_unit(b, m))
                if qi >= 2:
                    for tt in range(4):
                        fillers.append(
                            lambda q=qi - 2, t=tt: outproj_unit(q, t))
                if qi == NQT - 1:
                    for tt in range(4):
                        fillers.append(
                            lambda q=qi - 1, t=tt: outproj_unit(q, t))
                if qi + 1 < NQT:
                    for tt in range(4):
                        fillers.append(
                            lambda b=qi + 1, t=tt: v_unit(b, t))

                last = (qi == NQT - 1)
                for hp in range(2):
                    attention_pair(qi, hp)
                    if last and hp == 0:
                        # pair-0 output projection of the last block runs
                        # during pair-1's attention, into its own partial
                        for tt in range(4):
                            fillers.append(
                                lambda t=tt:
                                outproj_unit(NQT - 1, t, pairs=(0,)))
                # all remaining fillers must land before the next block's
                # attention (it needs the projections)
                pump(len(fillers))

            # tail: pair-1 output projection of the last block
            for tt in range(4):
                outproj_unit(NQT - 1, tt, pairs=(1,), ydst=y1,
                             yrow0=(NQT - 1) * QTILE)

    nc.compile()
    return nc


def make_in_maps(x, Wq, bq, Wkv, bkv, Wo, bo):
    import ml_dtypes

    bf = ml_dtypes.bfloat16
    x = np.asarray(x, np.float32)
    Wq = np.asarray(Wq, np.float32)
    bq = np.asarray(bq, np.float32)
    Wkv = np.asarray(Wkv, np.float32)
    bkv = np.asarray(bkv, np.float32)
    Wo = np.asarray(Wo, np.float32)

    Wk, Wv = Wkv[:D], Wkv[D:]
    bk, bv = bkv[:D], bkv[D:]

    # causal triangle keep-mask: mask[p, u] = 1 iff u >= p
    u = np.arange(128)[None, :]
    kk = np.arange(128)[:, None]
    mask = (u >= kk).astype(bf)

    in_maps = []
    for c in range(NCORES):
        b = c // (NCORES // B)
        hs = HPC * (c % (NCORES // B))
        rows = slice(hs * HD, hs * HD + HPC * HD)
        bq_c = bq[rows] * SCALE
        # bqz column m = the scaled q-bias for heads (2m, 2m+1) stacked
        bqzv = np.zeros((128, 2), np.float32)
        for m in range(2):
            bqzv[:, m] = bq_c[m * 128:(m + 1) * 128]
        wqkv_c = np.concatenate(
            [Wq[rows].T, Wk[rows].T, Wv[rows].T], axis=1)
        in_maps.append({
            "xT": np.ascontiguousarray(x[b].T).astype(bf),
            "wqkvT": np.ascontiguousarray(wqkv_c).astype(bf),
            "woT": np.ascontiguousarray(Wo[:, rows].T).astype(bf),
            "bqz": bqzv,
            "bk": np.ascontiguousarray(bk[rows]),
            "mask": mask,
        })
    return in_maps


_NC_CACHE = None


def _get_nc():
    global _NC_CACHE
    if _NC_CACHE is None:
        _NC_CACHE = build_kernel()
    return _NC_CACHE


def kernel(x, Wq, bq, Wkv, bkv, Wo, bo, _trace=False, _trace_kwargs=None):
    nc = _get_nc()
    in_maps = make_in_maps(x, Wq, bq, Wkv, bkv, Wo, bo)
    kwargs = {}
    if _trace:
        kwargs = dict(trace=True, trace_cores=list(range(NCORES)),
                      **(_trace_kwargs or {}))
    res = run_bass_kernel_spmd(nc, in_maps, core_ids=list(range(NCORES)),
                               **kwargs)
    out = np.zeros((B, N, D), np.float32)
    for c, r in enumerate(res.results):
        b = c // (NCORES // B)
        out[b] += np.asarray(r["y"], np.float32)
        out[b, (NQT - 1) * QTILE:] += np.asarray(r["y1"], np.float32)
    # bo plus the folded V-bias contribution (softmax rows sum to 1, so the
    # v-bias adds exactly bv @ Wo.T to every token)
    bv = np.asarray(bkv, np.float32)[D:]
    bo2 = np.asarray(bo, np.float32) + np.asarray(Wo, np.float32) @ bv
    out += bo2[None, None, :]
    if _trace:
        kernel.last_results = res
    return out
